# revision 14
# baseline (speedup 1.0000x reference)
"""FFM layer kernel for Trainium2 (8 NeuronCores, SPMD batch-parallel).

out = b + x @ W + 0.5 * (x^T A x - sum_i x_i^2 A_ii),
A[i,j] = <V[i, field(j)], V[j, field(i)]>.

v2 strategy: columns of x are sorted by field on the host so each field
group occupies a contiguous, 64-aligned partition range.  Per batch-tile
of 128 samples the tensor engine computes, per field group c1, the block
    T[b, c1, (c2,f)] = sum_{i in group c1} x[b,i] * V[i,c2,f]
(one matmul per group, x stationary, [V|W|0pad] streaming, N=328).
Groups are processed in strips of M=4 (one PSUM buffer = 4 banks).  The
scalar engine evicts each strip's rows TRANSPOSED into a c1-major SBUF
layout  tsbT[c1*R + 8*c2 + f] = T[c2, (c1,f)]  (3-free-dim scatter ACT),
with own-strip entries at scale 1.0 and later-block entries at scale 2.0.
One merged vector STT per strip (scale 0.5, in0 = PSUM strip rows x cols
[0, 8*(c0+M)), in1 = tsbT c1-blocks of the strip) accumulates
    cross pairs (0.5*2 = 1) + within pairs (0.5+0.5 = 1) + diag (0.5)
= 0.5 * x^T A x  into per-strip partials.  The diagonal correction
sum_i d_i x_i^2 is folded into a host-computed badj input (b - 0.5*dxx),
and the x@W term rides along as vrw column 320 (evicted at scale 2 ->
wsum * 0.5 in the epilogue).
"""

import sys

for _p in ("/opt/trn_rl_repo",):
    if _p not in sys.path:
        sys.path.insert(0, _p)

import numpy as np

import concourse.bass as bass
import concourse.tile as tile
from concourse import bacc, bass_utils, mybir

F32 = mybir.dt.float32
F16 = mybir.dt.float16

B, D, FIELDS, F = 4096, 2000, 40, 8
NCORES = 8
BS = B // NCORES          # batch shard per core (512)
BT = BS // 128            # batch tiles per core (4)
CF = FIELDS * F           # 320
VW = CF + 8               # 328 = V block + W column + 7 zero pad
R = VW                    # tsbT block stride (41 blocks: 40 c1 + W block)
M = 4                     # groups per strip (= PSUM banks per buffer)
NQ = FIELDS // M          # 10 strips


def _placement(counts):
    """Assign each field group a start row; groups <=64 rows go in 64-row
    slots, bigger groups take a whole 128-row tile alone."""
    offs = [0] * FIELDS
    pos = 0
    for c in range(FIELDS):
        n = int(counts[c])
        if n == 0:
            offs[c] = pos
            continue
        if n <= 64:
            if pos % 64 != 0:
                pos = (pos // 64 + 1) * 64
        else:
            if pos % 128 != 0:
                pos = (pos // 128 + 1) * 128
            assert n <= 128, f"field group of {n} > 128 rows unsupported"
        offs[c] = pos
        pos += n
    dp = ((pos + 127) // 128) * 128
    return offs, dp


def _ap(sliced, dims):
    """Re-dim a sliced [part, free] AP into [part, *dims] with explicit
    (stride, count) free dims; the slice supplies tensor + offset."""
    p = sliced.ap[0]
    return bass.AP(
        tensor=sliced.tensor,
        offset=sliced.offset,
        ap=[[p[0], p[1]]] + [[s, d] for s, d in dims],
    )


def _build(groups, ntiles, dp):
    """Build + compile the per-core program.  groups: list of (c, off, n)."""
    nc = bacc.Bacc(
        "TRN2",
        target_bir_lowering=False,
        debug=False,
        enable_asserts=False,
        num_devices=NCORES,
    )
    xt_d = nc.dram_tensor("xt", [128, ntiles * 512], F16, kind="ExternalInput").ap()
    vrw_d = nc.dram_tensor("vrw", [128, ntiles * VW], F16, kind="ExternalInput").ap()
    badj_d = nc.dram_tensor("badj", [128, BT], F32, kind="ExternalInput").ap()
    out_d = nc.dram_tensor("out", [BS, 1], F32, kind="ExternalOutput").ap()

    ginfo = {c: (off, n) for c, off, n in groups}

    with tile.TileContext(nc) as tc:
        with (
            tc.tile_pool(name="big", bufs=1) as big,
            tc.tile_pool(name="small", bufs=1) as small,
            tc.tile_pool(name="parts", bufs=2) as parts_pool,
            tc.tile_pool(name="tsbp", bufs=2) as tsb_pool,
            tc.tile_pool(name="scratch", bufs=2) as scratch_pool,
            tc.tile_pool(name="outp", bufs=2) as out_pool,
            tc.tile_pool(name="qp", bufs=2, space="PSUM") as qpool,
        ):
            xs = big.tile([128, BT * ntiles * 128], F16, tag="xs")
            vrw = big.tile([128, ntiles * VW], F16, tag="vrw")
            badj = small.tile([128, BT], F32)

            nc.gpsimd.dma_start(badj[:], badj_d[:, :])
            # xs is packed bt-major on the host: [128, (bt, tile, 128col)].
            # First chunk is tiny (bt0, tiles 0-2) so compute starts early.
            xb = ntiles * 128
            xbounds = [0, 3 * 128, xb, 2 * xb, 3 * xb, 4 * xb]
            for a, b_ in zip(xbounds, xbounds[1:]):
                nc.sync.dma_start(xs[:, a:b_], xt_d[:, a:b_])
            # vrw chunked on the gpsimd queue (keeps the scalar queue free
            # for the eviction ACTs)
            vbounds = [0, 2, 5, 9, 14, ntiles]
            for t0, t1 in zip(vbounds, vbounds[1:]):
                nc.gpsimd.dma_start(
                    vrw[:, t0 * VW : t1 * VW], vrw_d[:, t0 * VW : t1 * VW]
                )

            for bt in range(BT):
                partials = parts_pool.tile([128, NQ], F32, tag="partials")
                tsbT = tsb_pool.tile([128, (FIELDS + 1) * R], F16, tag="tsbT")
                pend_z2 = None
                for q in range(NQ):
                    c0 = q * M
                    qt = qpool.tile([128, M * 512], F32, tag="qt")
                    for c in range(c0, c0 + M):
                        off, n = ginfo[c]
                        slot = c - c0
                        assert n > 0
                        t = off // 128
                        lp = off % 128
                        if n <= 64:
                            base, kk = (lp // 64) * 64, 64
                        else:
                            base, kk = 0, 128
                        xcol = bt * ntiles * 128 + t * 128
                        nc.tensor.matmul(
                            qt[:, slot * 512 : slot * 512 + VW],
                            xs[base : base + kk, xcol : xcol + 128],
                            vrw[base : base + kk, t * VW : t * VW + VW],
                            start=True,
                            stop=True,
                        )
                    # 2-zone transposed eviction (scalar engine):
                    #   tsbT[c1*R + 8*c2 + f] = T[c2, (c1,f)]
                    # zone 1: c1 in strip, scale 1.0; zone 2: c1 >= c0+M, scale 2.0
                    # z2 is DEFERRED one strip (emitted at the next strip) so it
                    # executes hidden under the next strip's matmuls and the
                    # coalesced scalar-sem target for STT(q) lands after z1(q).
                    if pend_z2 is not None:
                        nc.scalar.activation(*pend_z2, scale=2.0)
                    in_z1 = _ap(qt[:, 8 * c0 :], [(512, M), (1, 8 * M)])
                    out_z1 = _ap(
                        tsbT[:, c0 * R + 8 * c0 :], [(8, M), (R, M), (1, F)]
                    )
                    nc.scalar.copy(out_z1, in_z1)
                    nblk2 = FIELDS + 1 - (c0 + M)
                    in_z2 = _ap(
                        qt[:, 8 * (c0 + M) :], [(512, M), (1, VW - 8 * (c0 + M))]
                    )
                    out_z2 = _ap(
                        tsbT[:, (c0 + M) * R + 8 * c0 :],
                        [(8, M), (R, nblk2), (1, F)],
                    )
                    pend_z2 = (
                        out_z2,
                        in_z2,
                        mybir.ActivationFunctionType.Copy,
                    )
                    # merged strip STT (vector): cross + within + diag
                    #   accum += sum (T[c1,(c2,f)] * 0.5) * s * T[c2,(c1,f)]
                    # emitted BEFORE z2 so its semaphore target is z1 only
                    w = 8 * (c0 + M)
                    in0 = _ap(qt[:, 0:], [(512, M), (1, w)])
                    in1 = _ap(tsbT[:, c0 * R :], [(R, M), (1, w)])
                    sc = scratch_pool.tile([128, M * CF], F16, tag="sc")
                    nc.vector.scalar_tensor_tensor(
                        _ap(sc[:, 0:], [(CF, M), (1, w)]),
                        in0,
                        0.5,
                        in1,
                        op0=mybir.AluOpType.mult,
                        op1=mybir.AluOpType.mult,
                        accum_out=partials[:, q : q + 1],
                    )
                if pend_z2 is not None:
                    nc.scalar.activation(*pend_z2, scale=2.0)
                    pend_z2 = None
                # epilogue: reduces on the scalar engine (ACT accum_out)
                wsum = out_pool.tile([128, 1], F32, tag="wsum")
                wtrash = out_pool.tile([128, FIELDS], F32, tag="wtrash")
                nc.scalar.activation(
                    wtrash[:],
                    _ap(tsbT[:, FIELDS * R :], [(8, FIELDS)]),
                    mybir.ActivationFunctionType.Copy,
                    accum_out=wsum[:],
                )
                psum_red = out_pool.tile([128, 1], F32, tag="psum_red")
                ptrash = out_pool.tile([128, NQ], F32, tag="ptrash")
                nc.scalar.activation(
                    ptrash[:],
                    partials[:],
                    mybir.ActivationFunctionType.Copy,
                    accum_out=psum_red[:],
                )
                # ob = wsum * 0.5 + psum_red  (W col was evicted at scale 2)
                ob = out_pool.tile([128, 1], F32, tag="ob")
                nc.vector.scalar_tensor_tensor(
                    ob[:],
                    wsum[:],
                    0.5,
                    psum_red[:],
                    op0=mybir.AluOpType.mult,
                    op1=mybir.AluOpType.add,
                )
                ob2 = out_pool.tile([128, 1], F32, tag="ob2")
                nc.vector.tensor_tensor(
                    ob2[:], ob[:], badj[:, bt : bt + 1], op=mybir.AluOpType.add
                )
                nc.sync.dma_start(out_d[bt * 128 : (bt + 1) * 128, :], ob2[:])

    nc.compile()
    return nc


def _host_prep(x, field_dict, b, W, V):
    x = np.ascontiguousarray(np.asarray(x, np.float32))
    fd = np.asarray(field_dict).astype(np.int64)
    W = np.asarray(W, np.float32)
    V = np.asarray(V, np.float32)
    b = np.asarray(b, np.float32)

    perm = np.argsort(fd, kind="stable")
    counts = np.bincount(fd[perm], minlength=FIELDS)
    offs, dp = _placement(counts)
    ntiles = dp // 128

    xt = np.zeros((dp, B), np.float32)
    vrw = np.zeros((dp, VW), np.float32)
    dpad = np.zeros((dp,), np.float32)
    groups = []
    src = 0
    for c in range(FIELDS):
        n = int(counts[c])
        o = offs[c]
        groups.append((c, o, n))
        if n:
            idx = perm[src : src + n]
            xt[o : o + n, :] = x[:, idx].T
            vrw[o : o + n, :CF] = V[idx].reshape(n, CF)
            vrw[o : o + n, CF] = W[idx, 0]
            dpad[o : o + n] = (V[idx, fd[idx], :] ** 2).sum(-1)
            src += n
    # badj = b - 0.5 * sum_i d_i x_i^2   (per sample)
    dxx = (dpad[:, None] * xt * xt).sum(0)          # [B]
    badj_full = (float(b[0]) - 0.5 * dxx).astype(np.float32)
    xt = xt.astype(np.float16)
    vrw = vrw.astype(np.float16)
    # pack to SBUF layout [128, ntiles * cols] (partition-major)
    xt = xt.reshape(ntiles, 128, B).transpose(1, 0, 2)
    vrw = np.ascontiguousarray(
        vrw.reshape(ntiles, 128, VW).transpose(1, 0, 2)
    ).reshape(128, ntiles * VW)
    return xt, vrw, badj_full, groups, ntiles, dp


def _in_maps(xt, vrw, badj_full):
    ntiles = xt.shape[1]
    in_maps = []
    for core in range(NCORES):
        sl = slice(core * BS, (core + 1) * BS)
        # bt-major xs: [128, (bt, tile, 128)]
        xc = xt[:, :, sl].reshape(128, ntiles, BT, 128)
        xc = np.ascontiguousarray(xc.transpose(0, 2, 1, 3)).reshape(128, -1)
        in_maps.append(
            {
                "xt": xc,
                "vrw": vrw,
                "badj": np.ascontiguousarray(badj_full[sl].reshape(BT, 128).T),
            }
        )
    return in_maps


def kernel(x, field_dict, b, W, V):
    xt, vrw, badj_full, groups, ntiles, dp = _host_prep(x, field_dict, b, W, V)
    nc = _build(groups, ntiles, dp)
    in_maps = _in_maps(xt, vrw, badj_full)
    res = bass_utils.run_bass_kernel_spmd(
        nc, in_maps, core_ids=list(range(NCORES))
    )
    out = np.concatenate([r["out"] for r in res.results], axis=0)
    return out.astype(np.float32)


if __name__ == "__main__":
    rng = np.random.default_rng(0)
    x = rng.standard_normal((B, D), dtype=np.float32)
    fd = rng.integers(0, FIELDS, size=(D,)).astype(np.int32)
    b = np.zeros((1,), np.float32)
    W = (rng.standard_normal((D, 1)) * 0.01).astype(np.float32)
    V = (rng.standard_normal((D, FIELDS, F)) * 0.01).astype(np.float32)
    out = kernel(x=x, field_dict=fd, b=b, W=W, V=V)
    print(out.shape, out.dtype, out[:4, 0])


# revision 16
# speedup vs baseline: 1.0607x; 1.0607x over previous
"""FFM layer kernel for Trainium2 (8 NeuronCores, SPMD batch-parallel).

out = b + x @ W + 0.5 * (x^T A x - sum_i x_i^2 A_ii),
A[i,j] = <V[i, field(j)], V[j, field(i)]>.

v2 strategy: columns of x are sorted by field on the host so each field
group occupies a contiguous, 64-aligned partition range.  Per batch-tile
of 128 samples the tensor engine computes, per field group c1, the block
    T[b, c1, (c2,f)] = sum_{i in group c1} x[b,i] * V[i,c2,f]
(one matmul per group, x stationary, [V|W|0pad] streaming, N=328).
Groups are processed in strips of M=4 (one PSUM buffer = 4 banks).  The
scalar engine evicts each strip's rows TRANSPOSED into a c1-major SBUF
layout  tsbT[c1*R + 8*c2 + f] = T[c2, (c1,f)]  (3-free-dim scatter ACT),
with own-strip entries at scale 1.0 and later-block entries at scale 2.0.
One merged vector STT per strip (scale 0.5, in0 = PSUM strip rows x cols
[0, 8*(c0+M)), in1 = tsbT c1-blocks of the strip) accumulates
    cross pairs (0.5*2 = 1) + within pairs (0.5+0.5 = 1) + diag (0.5)
= 0.5 * x^T A x  into per-strip partials.  The diagonal correction
sum_i d_i x_i^2 is folded into a host-computed badj input (b - 0.5*dxx),
and the x@W term rides along as vrw column 320 (evicted at scale 2 ->
wsum * 0.5 in the epilogue).
"""

import sys

for _p in ("/opt/trn_rl_repo",):
    if _p not in sys.path:
        sys.path.insert(0, _p)

import numpy as np

import concourse.bass as bass
import concourse.tile as tile
from concourse import bacc, bass_utils, mybir

F32 = mybir.dt.float32
F16 = mybir.dt.float16

B, D, FIELDS, F = 4096, 2000, 40, 8
NCORES = 8
BS = B // NCORES          # batch shard per core (512)
BT = BS // 128            # batch tiles per core (4)
CF = FIELDS * F           # 320
VW = CF + 8               # 328 = V block + W column + 7 zero pad
R = VW                    # tsbT block stride (41 blocks: 40 c1 + W block)
M = 4                     # groups per strip (= PSUM banks per buffer)
NQ = FIELDS // M          # 10 strips


def _placement(counts):
    """Assign each field group a start row; groups <=64 rows go in 64-row
    slots, bigger groups take a whole 128-row tile alone."""
    offs = [0] * FIELDS
    pos = 0
    for c in range(FIELDS):
        n = int(counts[c])
        if n == 0:
            offs[c] = pos
            continue
        if n <= 64:
            if pos % 64 != 0:
                pos = (pos // 64 + 1) * 64
        else:
            if pos % 128 != 0:
                pos = (pos // 128 + 1) * 128
            assert n <= 128, f"field group of {n} > 128 rows unsupported"
        offs[c] = pos
        pos += n
    dp = ((pos + 127) // 128) * 128
    return offs, dp


def _ap(sliced, dims):
    """Re-dim a sliced [part, free] AP into [part, *dims] with explicit
    (stride, count) free dims; the slice supplies tensor + offset."""
    p = sliced.ap[0]
    return bass.AP(
        tensor=sliced.tensor,
        offset=sliced.offset,
        ap=[[p[0], p[1]]] + [[s, d] for s, d in dims],
    )


def _build(groups, ntiles, dp):
    """Build + compile the per-core program.  groups: list of (c, off, n)."""
    nc = bacc.Bacc(
        "TRN2",
        target_bir_lowering=False,
        debug=False,
        enable_asserts=False,
        num_devices=NCORES,
    )
    xt_d = nc.dram_tensor("xt", [128, ntiles * 512], F16, kind="ExternalInput").ap()
    vrw_d = nc.dram_tensor("vrw", [128, ntiles * VW], F16, kind="ExternalInput").ap()
    badj_d = nc.dram_tensor("badj", [128, BT], F32, kind="ExternalInput").ap()
    out_d = nc.dram_tensor("out", [BS, 1], F32, kind="ExternalOutput").ap()

    ginfo = {c: (off, n) for c, off, n in groups}

    with tile.TileContext(nc) as tc:
        with (
            tc.tile_pool(name="big", bufs=1) as big,
            tc.tile_pool(name="small", bufs=1) as small,
            tc.tile_pool(name="parts", bufs=2) as parts_pool,
            tc.tile_pool(name="tsbp", bufs=2) as tsb_pool,
            tc.tile_pool(name="scratch", bufs=2) as scratch_pool,
            tc.tile_pool(name="outp", bufs=2) as out_pool,
            tc.tile_pool(name="qp", bufs=2, space="PSUM") as qpool,
        ):
            xs = big.tile([128, BT * ntiles * 128], F16, tag="xs")
            vrw = big.tile([128, ntiles * VW], F16, tag="vrw")
            badj = small.tile([128, BT], F32)

            nc.gpsimd.dma_start(badj[:], badj_d[:, :])
            # xs is packed bt-major on the host: [128, (bt, tile, 128col)].
            # First chunk is tiny (bt0, tiles 0-2) so compute starts early.
            xb = ntiles * 128
            xbounds = [0, 3 * 128, xb, 2 * xb, 3 * xb, 4 * xb]
            for a, b_ in zip(xbounds, xbounds[1:]):
                nc.sync.dma_start(xs[:, a:b_], xt_d[:, a:b_])
            # vrw chunked on the gpsimd queue (keeps the scalar queue free
            # for the eviction ACTs)
            vbounds = [0, 2, 5, 9, 14, ntiles]
            for t0, t1 in zip(vbounds, vbounds[1:]):
                nc.gpsimd.dma_start(
                    vrw[:, t0 * VW : t1 * VW], vrw_d[:, t0 * VW : t1 * VW]
                )

            for bt in range(BT):
                partials = parts_pool.tile([128, 2 * NQ], F32, tag="partials")
                nc.vector.memset(partials[:, 0:1], 0.0)
                tsbT = tsb_pool.tile([128, (FIELDS + 1) * R], F16, tag="tsbT")
                for q in range(NQ):
                    c0 = q * M
                    qt = qpool.tile([128, M * 512], F32, tag="qt")
                    for c in range(c0, c0 + M):
                        off, n = ginfo[c]
                        slot = c - c0
                        assert n > 0
                        t = off // 128
                        lp = off % 128
                        if n <= 64:
                            base, kk = (lp // 64) * 64, 64
                        else:
                            base, kk = 0, 128
                        xcol = bt * ntiles * 128 + t * 128
                        nc.tensor.matmul(
                            qt[:, slot * 512 : slot * 512 + VW],
                            xs[base : base + kk, xcol : xcol + 128],
                            vrw[base : base + kk, t * VW : t * VW + VW],
                            start=True,
                            stop=True,
                        )
                    # 2-zone transposed eviction:
                    #   tsbT[c1*R + 8*c2 + f] = T[c2, (c1,f)]
                    # zone 2 (c1 >= c0+M, scale 2.0) on the scalar engine.
                    nblk2 = FIELDS + 1 - (c0 + M)
                    in_z2 = _ap(
                        qt[:, 8 * (c0 + M) :], [(512, M), (1, VW - 8 * (c0 + M))]
                    )
                    out_z2 = _ap(
                        tsbT[:, (c0 + M) * R + 8 * c0 :],
                        [(8, M), (R, nblk2), (1, F)],
                    )
                    nc.scalar.activation(
                        out_z2,
                        in_z2,
                        mybir.ActivationFunctionType.Copy,
                        scale=2.0,
                    )
                    # cross STT (vector): pairs (c1 in strip, c2 < c0), weight
                    # 0.5 * 2.0 = 1.  Depends only on MMs(q) + earlier z2 --
                    # NOT on this strip's z1 -- so it starts immediately.
                    sc = scratch_pool.tile([128, M * CF], F16, tag="sc")
                    if q > 0:
                        w = 8 * c0
                        nc.vector.scalar_tensor_tensor(
                            _ap(sc[:, 0:], [(CF, M), (1, w)]),
                            _ap(qt[:, 0:], [(512, M), (1, w)]),
                            0.5,
                            _ap(tsbT[:, c0 * R :], [(R, M), (1, w)]),
                            op0=mybir.AluOpType.mult,
                            op1=mybir.AluOpType.mult,
                            accum_out=partials[:, 2 * q : 2 * q + 1],
                        )
                    # zone 1 (own strip, scale 1.0) as a vector CAST, then the
                    # small within+diag STT (weights 0.5+0.5=1 pairs, 0.5 diag)
                    in_z1 = _ap(qt[:, 8 * c0 :], [(512, M), (1, 8 * M)])
                    out_z1 = _ap(
                        tsbT[:, c0 * R + 8 * c0 :], [(8, M), (R, M), (1, F)]
                    )
                    nc.vector.tensor_copy(out_z1, in_z1)
                    nc.vector.scalar_tensor_tensor(
                        _ap(sc[:, CF * M - 8 * M * M :], [(8 * M, M), (1, 8 * M)]),
                        _ap(qt[:, 8 * c0 :], [(512, M), (1, 8 * M)]),
                        0.5,
                        _ap(tsbT[:, c0 * R + 8 * c0 :], [(R, M), (1, 8 * M)]),
                        op0=mybir.AluOpType.mult,
                        op1=mybir.AluOpType.mult,
                        accum_out=partials[:, 2 * q + 1 : 2 * q + 2],
                    )
                # epilogue: reduces on the scalar engine (ACT accum_out)
                wsum = out_pool.tile([128, 1], F32, tag="wsum")
                wtrash = out_pool.tile([128, FIELDS], F32, tag="wtrash")
                nc.scalar.activation(
                    wtrash[:],
                    _ap(tsbT[:, FIELDS * R :], [(8, FIELDS)]),
                    mybir.ActivationFunctionType.Copy,
                    accum_out=wsum[:],
                )
                psum_red = out_pool.tile([128, 1], F32, tag="psum_red")
                ptrash = out_pool.tile([128, 2 * NQ], F32, tag="ptrash")
                nc.scalar.activation(
                    ptrash[:],
                    partials[:],
                    mybir.ActivationFunctionType.Copy,
                    accum_out=psum_red[:],
                )
                # ob = wsum * 0.5 + psum_red  (W col was evicted at scale 2)
                ob = out_pool.tile([128, 1], F32, tag="ob")
                nc.vector.scalar_tensor_tensor(
                    ob[:],
                    wsum[:],
                    0.5,
                    psum_red[:],
                    op0=mybir.AluOpType.mult,
                    op1=mybir.AluOpType.add,
                )
                ob2 = out_pool.tile([128, 1], F32, tag="ob2")
                nc.vector.tensor_tensor(
                    ob2[:], ob[:], badj[:, bt : bt + 1], op=mybir.AluOpType.add
                )
                nc.sync.dma_start(out_d[bt * 128 : (bt + 1) * 128, :], ob2[:])

    nc.compile()
    return nc


def _host_prep(x, field_dict, b, W, V):
    x = np.ascontiguousarray(np.asarray(x, np.float32))
    fd = np.asarray(field_dict).astype(np.int64)
    W = np.asarray(W, np.float32)
    V = np.asarray(V, np.float32)
    b = np.asarray(b, np.float32)

    perm = np.argsort(fd, kind="stable")
    counts = np.bincount(fd[perm], minlength=FIELDS)
    offs, dp = _placement(counts)
    ntiles = dp // 128

    xt = np.zeros((dp, B), np.float32)
    vrw = np.zeros((dp, VW), np.float32)
    dpad = np.zeros((dp,), np.float32)
    groups = []
    src = 0
    for c in range(FIELDS):
        n = int(counts[c])
        o = offs[c]
        groups.append((c, o, n))
        if n:
            idx = perm[src : src + n]
            xt[o : o + n, :] = x[:, idx].T
            vrw[o : o + n, :CF] = V[idx].reshape(n, CF)
            vrw[o : o + n, CF] = W[idx, 0]
            dpad[o : o + n] = (V[idx, fd[idx], :] ** 2).sum(-1)
            src += n
    # badj = b - 0.5 * sum_i d_i x_i^2   (per sample)
    dxx = (dpad[:, None] * xt * xt).sum(0)          # [B]
    badj_full = (float(b[0]) - 0.5 * dxx).astype(np.float32)
    xt = xt.astype(np.float16)
    vrw = vrw.astype(np.float16)
    # pack to SBUF layout [128, ntiles * cols] (partition-major)
    xt = xt.reshape(ntiles, 128, B).transpose(1, 0, 2)
    vrw = np.ascontiguousarray(
        vrw.reshape(ntiles, 128, VW).transpose(1, 0, 2)
    ).reshape(128, ntiles * VW)
    return xt, vrw, badj_full, groups, ntiles, dp


def _in_maps(xt, vrw, badj_full):
    ntiles = xt.shape[1]
    in_maps = []
    for core in range(NCORES):
        sl = slice(core * BS, (core + 1) * BS)
        # bt-major xs: [128, (bt, tile, 128)]
        xc = xt[:, :, sl].reshape(128, ntiles, BT, 128)
        xc = np.ascontiguousarray(xc.transpose(0, 2, 1, 3)).reshape(128, -1)
        in_maps.append(
            {
                "xt": xc,
                "vrw": vrw,
                "badj": np.ascontiguousarray(badj_full[sl].reshape(BT, 128).T),
            }
        )
    return in_maps


def kernel(x, field_dict, b, W, V):
    xt, vrw, badj_full, groups, ntiles, dp = _host_prep(x, field_dict, b, W, V)
    nc = _build(groups, ntiles, dp)
    in_maps = _in_maps(xt, vrw, badj_full)
    res = bass_utils.run_bass_kernel_spmd(
        nc, in_maps, core_ids=list(range(NCORES))
    )
    out = np.concatenate([r["out"] for r in res.results], axis=0)
    return out.astype(np.float32)


if __name__ == "__main__":
    rng = np.random.default_rng(0)
    x = rng.standard_normal((B, D), dtype=np.float32)
    fd = rng.integers(0, FIELDS, size=(D,)).astype(np.int32)
    b = np.zeros((1,), np.float32)
    W = (rng.standard_normal((D, 1)) * 0.01).astype(np.float32)
    V = (rng.standard_normal((D, FIELDS, F)) * 0.01).astype(np.float32)
    out = kernel(x=x, field_dict=fd, b=b, W=W, V=V)
    print(out.shape, out.dtype, out[:4, 0])
